# revision 1
# baseline (speedup 1.0000x reference)
"""Trainium2 Bass kernel for block-local (chunked) attention.

Problem: x:(4,4096,1024) f32. qkv = x @ w_qkv.T; block-local attention with
chunk=64 inside each head (16 heads, dim_head 64); out proj w_out + b_out.

Strategy (8 cores, SPMD):
  - Shard the 16384 flattened tokens into 8 contiguous shards of 2048
    (chunk-aligned, so blocks never cross shards).
  - Host pre-transposes x and the weights so every DMA is contiguous and
    every matmul operand has the contraction dim on partitions.
  - Per core: qkv projection (bf16 matmuls at N=1024, fp32 PSUM
    accumulate), block attention with the softmax reduction done ON the
    partition axis via a block-identity matmul (sum + broadcast in one PE
    op), final projection, pipelined with the Tile framework.

Layouts on device (P=128 partitions always first):
  xT     [128, 8, T]     bf16   xT[p,kc,t]  = x_shard[t, kc*128+p]
  wqkvT  [128, 8, 3072]  bf16   [p,kc,f]    = w_qkv[f, kc*128+p]
  woutT  [128, 8, 1024]  bf16   [p,hp,e]    = w_out[e, hp*128+p]
  consts [128, 128]      bf16   [:, :64]=upper-half ones, [:, 64:]=lower-half
  yT     [8, 128, T]     f32    yT[ec,p,t]  = y_shard[t, ec*128+p]

HW gotcha baked in below: matmuls whose stationary operands live at
different base partitions (row groups 0 vs 64) must never target the same
PSUM bank — that crashes the device. Scores matmuls are therefore grouped
by head parity into separate PSUM tiles.
"""

import os
import sys

for _p in ("/opt/trn_rl_repo", "/root/.axon_site/_ro/trn_rl_repo"):
    if os.path.isdir(_p) and _p not in sys.path:
        sys.path.append(_p)

import numpy as np
import ml_dtypes

import concourse.bass as bass
from concourse import bacc
from concourse import mybir
from concourse import tile

BF16 = mybir.dt.bfloat16
F32 = mybir.dt.float32
BF16_NP = ml_dtypes.bfloat16

P = 128
KC = 8            # contraction chunks for dim=1024
HEADS = 16
DH = 64
CHUNK = 64
INNER = HEADS * DH            # 1024
DIM = 1024
N_CORES = 8
ST = 128                      # tokens per attention subtile
SCALE = DH ** -0.5


def build_body(tc, yT, xT, wqkvT, woutT, consts, T):
    """Emit the whole per-core program into TileContext tc."""
    nc = tc.nc
    GT = min(512, T)          # tokens per group
    G = T // GT
    NST = GT // ST            # subtiles per group
    import contextlib
    ctx = contextlib.ExitStack()

    # --- SBUF pools -----------------------------------------------------
    wpool = ctx.enter_context(tc.tile_pool(name="w", bufs=1))
    xpool = ctx.enter_context(tc.tile_pool(name="x", bufs=2))
    qkpool = ctx.enter_context(tc.tile_pool(name="qk", bufs=2))
    vpool = ctx.enter_context(tc.tile_pool(name="v", bufs=3))
    epool = ctx.enter_context(tc.tile_pool(name="e", bufs=2))
    rbpool = ctx.enter_context(tc.tile_pool(name="rb", bufs=2))
    enpool = ctx.enter_context(tc.tile_pool(name="en", bufs=1))
    apool = ctx.enter_context(tc.tile_pool(name="a", bufs=2))
    ypool = ctx.enter_context(tc.tile_pool(name="y", bufs=3))

    # --- PSUM pools: 2 + 2 + 2 + 1 + 1 = 8 banks ------------------------
    ps_big = ctx.enter_context(tc.tile_pool(name="ps_big", bufs=2, space="PSUM"))
    ps_v = ctx.enter_context(tc.tile_pool(name="ps_v", bufs=2, space="PSUM"))
    ps_sc = ctx.enter_context(tc.tile_pool(name="ps_sc", bufs=2, space="PSUM"))
    ps_sm = ctx.enter_context(tc.tile_pool(name="ps_sm", bufs=1, space="PSUM"))
    ps_at = ctx.enter_context(tc.tile_pool(name="ps_at", bufs=1, space="PSUM"))

    # --- first group's x load gets queue priority: the very first matmul
    # needs x chunk 0 + wqkv chunk 0 only (~1 MiB), not the whole 10 MiB
    # of startup traffic.
    x_first = xpool.tile([P, KC * (GT if T >= 512 else T)], BF16, tag="x",
                         name="x_first")
    GT0 = min(512, T)
    for kc in range(KC):
        nc.sync.dma_start(
            x_first[:, kc * GT0:(kc + 1) * GT0], xT[:, kc, 0:GT0]
        )

    # --- resident weights ----------------------------------------------
    # Chunked per-kc DMAs so the first projection matmuls only wait for the
    # first chunk instead of the whole 6 MiB weight load.
    wqkv_sb = wpool.tile([P, KC * 3072], BF16, tag="wqkv")
    for kc in range(KC):
        nc.sync.dma_start(
            wqkv_sb[:, kc * 3072:(kc + 1) * 3072], wqkvT[:, kc, :]
        )
    consts_sb = wpool.tile([P, 128], BF16, tag="consts")
    nc.sync.dma_start(consts_sb[:], consts[:, :])
    # w_out is only needed by the first final-projection (~1/3 into the
    # kernel) — defer its DMA so the startup loads get the full bandwidth.
    wout_sb = wpool.tile([P, KC * 1024], BF16, tag="wout")
    wout_loaded = [False]

    def load_wout():
        if not wout_loaded[0]:
            nc.sync.dma_start(
                wout_sb[:].rearrange("p (k e) -> p k e", k=KC), woutT[:, :, :]
            )
            wout_loaded[0] = True

    # --- persistent pre-zeroed normalized-probs tiles -------------------
    # En garbage regions (cross-chunk blocks) stay zero forever; only the
    # valid block-diagonal regions are rewritten each iteration, letting the
    # attn@v matmul contract over the full 128 partitions in one shot.
    en_tiles = []
    for i in range(4):
        t = enpool.tile([P, 8 * 128], BF16, tag=f"en{i}", name=f"en{i}")
        nc.gpsimd.memset(t[:], 0.0)
        en_tiles.append(t)

    half_idx = 0
    for g in range(G):
        # ---- x tile for this group ------------------------------------
        if g == 0:
            x_t = x_first
        else:
            x_t = xpool.tile([P, KC * GT], BF16, tag="x")
            for kc in range(KC):
                nc.sync.dma_start(
                    x_t[:, kc * GT:(kc + 1) * GT],
                    xT[:, kc, g * GT:(g + 1) * GT],
                )

        # ---- q/k projections: out layout [feat, tok] -------------------
        q_sb = qkpool.tile([P, 8 * GT], BF16, tag="q")
        k_sb = qkpool.tile([P, 8 * GT], BF16, tag="k")
        attn_sb = apool.tile([P, NST * 8 * 128], BF16, tag="attn",
                             name=f"attn_{g}")
        for fc in range(16):
            qk_ps = ps_big.tile([P, GT], F32, tag="big")
            for kc in range(KC):
                nc.tensor.matmul(
                    qk_ps[:],
                    lhsT=wqkv_sb[:, kc * 3072 + fc * 128: kc * 3072 + fc * 128 + 128],
                    rhs=x_t[:, kc * GT:(kc + 1) * GT],
                    start=(kc == 0),
                    stop=(kc == KC - 1),
                )
            dst = q_sb if fc < 8 else k_sb
            sl = dst[:, (fc % 8) * GT:(fc % 8 + 1) * GT]
            if fc % 2 == 0:
                nc.vector.tensor_copy(sl, qk_ps[:])
            else:
                nc.scalar.copy(sl, qk_ps[:])

        # ---- per 128-token subtile: v projection + attention -----------
        for st in range(NST):
            # v projection: out layout [tok, feat], one N=1024 matmul per kc
            v_sb = vpool.tile([P, INNER], BF16, tag="v", name=f"v_{g}_{st}")
            for half in range(2):
                v_ps = ps_v.tile([P, 512], F32, tag="v")
                for kc in range(KC):
                    nc.tensor.matmul(
                        v_ps[:],
                        lhsT=x_t[:, kc * GT + st * ST: kc * GT + st * ST + ST],
                        rhs=wqkv_sb[:, kc * 3072 + 2048 + half * 512:
                                    kc * 3072 + 2048 + (half + 1) * 512],
                        start=(kc == 0),
                        stop=(kc == KC - 1),
                    )
                nc.vector.tensor_copy(v_sb[:, half * 512:(half + 1) * 512], v_ps[:])

            for hh in range(2):       # halves: heads hh*8 .. hh*8+7
                # scoresT[tk, tq] per head: stationary kT, moving qT.
                # E column block j holds head h = hh*8 + 2*(j%4) + (j//4):
                # even heads (stationary base partition 0) fill sc_a, odd
                # heads (base 64) fill sc_b — mixing row groups within one
                # PSUM bank is fatal on TRN2 hardware.
                sc_a = ps_sc.tile([P, 512], F32, tag="sc")
                sc_b = ps_sc.tile([P, 512], F32, tag="sc")
                for j in range(8):
                    h = hh * 8 + 2 * (j % 4) + (j // 4)
                    fc = h // 2
                    hb = (h % 2) * 64
                    sc = sc_a if j < 4 else sc_b
                    nc.tensor.matmul(
                        sc[:, (j % 4) * 128:(j % 4 + 1) * 128],
                        lhsT=k_sb[hb:hb + 64, fc * GT + st * ST: fc * GT + st * ST + ST],
                        rhs=q_sb[hb:hb + 64, fc * GT + st * ST: fc * GT + st * ST + ST],
                        start=True,
                        stop=True,
                    )
                # exp (scale folded in); no max-subtraction needed: scores~N(0,1)
                e_sb = epool.tile([P, 8 * 128], BF16, tag="e")
                nc.scalar.activation(
                    e_sb[:, 0:512], sc_a[:],
                    mybir.ActivationFunctionType.Exp, scale=SCALE,
                )
                nc.scalar.activation(
                    e_sb[:, 512:1024], sc_b[:],
                    mybir.ActivationFunctionType.Exp, scale=SCALE,
                )
                # denominators: block-identity matmul sums over tk (partition
                # axis) AND broadcasts the result to all 64 partitions of the
                # matching chunk. Garbage cross-chunk scores are excluded by
                # the zeros in the stationary.
                e3 = e_sb[:].rearrange("p (h q) -> p h q", h=8)
                sm_ps = ps_sm.tile([P, 512], F32, tag="sm")
                nc.tensor.matmul(
                    sm_ps[0:64, :],
                    lhsT=consts_sb[:, 0:64],
                    rhs=e3[:, :, 0:64],
                    start=True, stop=True,
                )
                nc.tensor.matmul(
                    sm_ps[64:128, :],
                    lhsT=consts_sb[:, 64:128],
                    rhs=e3[:, :, 64:128],
                    start=True, stop=True,
                )
                rb = rbpool.tile([P, 512], F32, tag="rb")
                nc.vector.reciprocal_approx_fast(out=rb[:], in_=sm_ps[:])
                # normalize: En = E * rb  (valid block-diagonal regions only)
                en = en_tiles[half_idx % 4]
                half_idx += 1
                en3 = en[:].rearrange("p (h q) -> p h q", h=8)
                rb3 = rb[:].rearrange("p (h q) -> p h q", h=8)
                nc.vector.tensor_mul(
                    en3[0:64, :, 0:64], e3[0:64, :, 0:64], rb3[0:64, :, :]
                )
                nc.vector.tensor_mul(
                    en3[64:128, :, 64:128], e3[64:128, :, 64:128], rb3[64:128, :, :]
                )
                # attn @ v : outT[d(head), tok] — full-partition contract,
                # En's zeros kill the cross-chunk terms.
                at_ps = ps_at.tile([P, 512], F32, tag="at")
                for j in range(8):
                    h = hh * 8 + 2 * (j % 4) + (j // 4)
                    hb = (h % 2) * 64
                    nc.tensor.matmul(
                        at_ps[hb:hb + 64, (j % 4) * 128:(j % 4 + 1) * 128],
                        lhsT=v_sb[:, h * 64:(h + 1) * 64],
                        rhs=en[:, j * 128:(j + 1) * 128],
                        start=True, stop=True,
                    )
                nc.scalar.copy(
                    attn_sb[:, st * 1024 + hh * 512: st * 1024 + (hh + 1) * 512],
                    at_ps[:],
                )

        # ---- final projection ------------------------------------------
        # For the last group there is no following work to hide the
        # attention→final serialization, so split it into two token halves:
        # the first half only needs subtiles 0..NST/2-1 and overlaps the
        # rest of the attention.
        load_wout()
        a3 = attn_sb[:].rearrange("p (s h t) -> p s h t", s=NST, h=8)
        halves = ((0, NST // 2), (NST // 2, NST)) if (g == G - 1 and NST > 1) \
            else ((0, NST),)
        for s0, s1 in halves:
            ht = (s1 - s0) * ST
            for ec in range(8):
                f_ps = ps_big.tile([P, GT], F32, tag="big")
                for hp in range(KC):
                    nc.tensor.matmul(
                        f_ps[:, 0:ht],
                        lhsT=wout_sb[:, hp * 1024 + ec * 128: hp * 1024 + ec * 128 + 128],
                        rhs=a3[:, s0:s1, hp, :],
                        start=(hp == 0),
                        stop=(hp == KC - 1),
                    )
                y_sb = ypool.tile([P, GT], F32, tag="y")
                if ec % 2 == 0:
                    nc.vector.tensor_copy(y_sb[:, 0:ht], f_ps[:, 0:ht])
                else:
                    nc.scalar.copy(y_sb[:, 0:ht], f_ps[:, 0:ht])
                nc.sync.dma_start(
                    yT[ec][:, g * GT + s0 * ST: g * GT + s1 * ST], y_sb[:, 0:ht]
                )

    ctx.close()


def build_nc(T):
    nc = bacc.Bacc("TRN2", target_bir_lowering=False, debug=False)
    xT = nc.dram_tensor("xT", [P, KC, T], BF16, kind="ExternalInput").ap()
    wqkvT = nc.dram_tensor("wqkvT", [P, KC, 3072], BF16, kind="ExternalInput").ap()
    woutT = nc.dram_tensor("woutT", [P, KC, 1024], BF16, kind="ExternalInput").ap()
    consts = nc.dram_tensor("consts", [P, 128], BF16, kind="ExternalInput").ap()
    yT = nc.dram_tensor("yT", [KC, P, T], F32, kind="ExternalOutput").ap()
    with tile.TileContext(nc) as tc:
        build_body(tc, yT, xT, wqkvT, woutT, consts, T)
    nc.compile()
    return nc


def make_consts():
    c = np.zeros((P, 128), dtype=BF16_NP)
    c[0:64, 0:64] = 1
    c[64:128, 64:128] = 1
    return c


def prep_inputs(x, w_qkv, w_out, T):
    """Host-side shard + transpose + cast. Returns in_maps list for SPMD."""
    tok = x.shape[0] * x.shape[1]
    flat = np.ascontiguousarray(x.reshape(tok, DIM))
    wqkvT = np.ascontiguousarray(
        w_qkv.T.reshape(KC, P, 3072).transpose(1, 0, 2)
    ).astype(BF16_NP)
    woutT = np.ascontiguousarray(
        w_out.T.reshape(KC, P, 1024).transpose(1, 0, 2)
    ).astype(BF16_NP)
    consts = make_consts()
    n_cores = tok // T
    in_maps = []
    for c in range(n_cores):
        shard = flat[c * T:(c + 1) * T]           # [T, 1024]
        xTc = np.ascontiguousarray(
            shard.T.reshape(KC, P, T).transpose(1, 0, 2)
        ).astype(BF16_NP)
        in_maps.append({"xT": xTc, "wqkvT": wqkvT, "woutT": woutT,
                        "consts": consts})
    return in_maps


def postprocess(results, b_out, bshape, T):
    outs = []
    for r in results:
        yT = np.asarray(r["yT"], dtype=np.float32)    # [8, 128, T]
        outs.append(yT.reshape(DIM, T).T)             # [T, 1024]
    y = np.concatenate(outs, axis=0)                  # [tok, 1024]
    y = y + np.asarray(b_out, dtype=np.float32)[None, :]
    return y.reshape(*bshape, DIM)


_CACHED = {}


def kernel(x, w_qkv, w_out, b_out):
    from concourse.bass_utils import run_bass_kernel_spmd

    x = np.asarray(x)
    b, n, _ = x.shape
    T = (b * n) // N_CORES
    if T not in _CACHED:
        _CACHED[T] = build_nc(T)
    nc = _CACHED[T]
    in_maps = prep_inputs(x, np.asarray(w_qkv), np.asarray(w_out), T)
    res = run_bass_kernel_spmd(nc, in_maps, list(range(N_CORES)))
    return postprocess(res.results, b_out, (b, n), T)


if __name__ == "__main__":
    nc = build_nc(2048)
    print("built ok")

